# revision 62
# baseline (speedup 1.0000x reference)
# Trainium2 Bass kernel for nn_AttentionLayer_69380901699611.
#
# Full-input contract: kernel(**inputs) takes the unsharded numpy inputs and
# returns the full [B, F, HIDDEN] output. Internally the work is sharded over
# 8 NeuronCores as (batch x head-group): core c handles batch c//4 and heads
# [4*(c%4), 4*(c%4)+4). Each core computes a partial output projection over
# its 4 heads; the host sums the 4 partials per batch.
#
# v5 design — ACT (exp) is the roofline (128 exps of N=1024 at ~1038ns =
# ~133us); every other engine is scheduled to stay strictly below it:
#   - q/k/v projections bf16 (PE has slack; fp8 here costs accuracy).
#   - scores: fp8-e4m3 DoubleRow, one matmul per (head, T-tile): head h's
#     64-deep contraction is 2 k-tiles of 32 at partitions 32h..32h+31
#     (host pre-orders weight columns into A=d0-31 / B=d32-63 blocks).
#     Weights are scaled x32 so e4m3 has mantissa; the exp's scale immediate
#     2^-13 = 1/(32*32*sqrt(64)) undoes it (wo is divided by 32).
#   - softmax: exp on ACT (PSUM->SBUF bf16), *exp(bias) split DVE (heads
#     0-2, 2x mode) + GPSIMD (head 3). No max-subtraction: |logits/8| < ~12.
#   - attention accumulates in [F-part 128, head, 64] layout: 4 F-tiles in
#     2 PSUM banks (2 per bank); denominators via N=1 matmuls (pt.T @ ones)
#     into a shared bank. PSUM "start" bumps a per-bank epoch (stale-tag
#     writes overwrite), so each bank gets exactly ONE start/stop per chunk.
#   - normalize = reciprocal + one mult per F-tile; PE-transpose into attnT
#     via a dedicated finish bank; bf16 output projection, DMA'd per F-tile.
#   - PSUM: scores 2x2 banks + value-accum 2 + denom 1 + finish 1 = 8.
#   - Schedule: first-chunk DMAs split per k-tile so PE starts at ~2us;
#     v-projection pieces and late q-projections are spread into per-tt
#     slots (budget <= ~1.7us under the 2.08us/tt ACT pace); chunk 2's
#     consume is compressed 2-per-slot so chunk 3's consume starts
#     mid-iteration and the tail stays short.

import numpy as np

B, F, T, C = 2, 2048, 2048, 1024
HEADS, DEPTH = 16, 64
N_CORES = 8
HG = 4   # head-groups; heads per group = HEADS // HG = 4
WSC = 32.0  # host-side weight scale for e4m3


def build_attention_nc(C=1024, F=2048, T=2048, NHEADS=4, H=64, fc_w=512):
    import concourse.tile as tile
    import concourse.mybir as mybir
    from concourse import bacc

    P = 128
    NH = NHEADS * H          # 256
    KC = C // P              # 8 k-tiles for the projections
    NFC = F // fc_w          # 4 F chunks
    NTT = T // P             # 16 T tiles
    FPC = fc_w // P          # 4 F tiles per chunk
    f32 = mybir.dt.float32
    bf16 = mybir.dt.bfloat16
    f8e4 = mybir.dt.float8e4
    Exp = mybir.ActivationFunctionType.Exp
    Mult = mybir.AluOpType.mult
    DR = mybir.MatmulPerfMode.DoubleRow
    ESCALE = 1.0 / (WSC * WSC * H ** 0.5)  # 2^-13

    nc = bacc.Bacc("TRN2", target_bir_lowering=False, debug=False, name="attn69")

    qT_d = nc.dram_tensor("qT", [C, F], bf16, kind="ExternalInput")
    sT_d = nc.dram_tensor("sT", [C, T], bf16, kind="ExternalInput")
    eb_d = nc.dram_tensor("ebT", [T, F], bf16, kind="ExternalInput")
    # wq/wk columns: [A: h0 d0-31 | h1 d0-31 | ... | B: h0 d32-63 | ...]
    wq_d = nc.dram_tensor("wq", [C, 2, P], bf16, kind="ExternalInput")
    wk_d = nc.dram_tensor("wk", [C, 2, P], bf16, kind="ExternalInput")
    wv_d = nc.dram_tensor("wv", [C, NH], bf16, kind="ExternalInput")
    wo_d = nc.dram_tensor("wo", [NH, C], bf16, kind="ExternalInput")
    sth_d = nc.dram_tensor("sth", [P, KC * P], bf16, kind="ExternalInput")
    id_d = nc.dram_tensor("ident", [P, P], f32, kind="ExternalInput")
    out_d = nc.dram_tensor("out_p", [F, C], f32, kind="ExternalOutput")

    with tile.TileContext(nc) as tc:
        with (
            tc.tile_pool(name="constp", bufs=1) as constp,
            tc.tile_pool(name="persist", bufs=1) as persist,
            tc.tile_pool(name="qap", bufs=2) as qap,
            tc.tile_pool(name="sap", bufs=4) as sap,
            tc.tile_pool(name="biasp", bufs=6) as biasp,
            tc.tile_pool(name="ptp", bufs=18) as ptp,
            tc.tile_pool(name="flatp", bufs=2) as flatp,
            tc.tile_pool(name="smallp", bufs=4) as smallp,
            tc.tile_pool(name="outp", bufs=6) as outp,
            tc.tile_pool(name="psS", bufs=2, space="PSUM") as psS,     # 4 banks
            tc.tile_pool(name="vaccp", bufs=2, space="PSUM") as vaccp,  # 2 banks
            tc.tile_pool(name="denp", bufs=1, space="PSUM") as denp,    # 1 bank
            tc.tile_pool(name="finp", bufs=1, space="PSUM") as finp,    # 1 bank
        ):
            # weight tiles; only wq/wk DMA'd up front (split per k-tile),
            # the rest deferred off the critical path
            wq_sb = constp.tile([P, KC, 2, P], bf16, name="wq_sb")
            wk_sb = constp.tile([P, KC, 2, P], bf16, name="wk_sb")
            wv_sb = constp.tile([P, KC, NH], bf16, name="wv_sb")
            wo_sb = constp.tile([P, 2, C], bf16, name="wo_sb")
            ident = constp.tile([P, P], f32, name="ident")
            ones1 = constp.tile([P, 1], bf16, name="ones1")
            sa_head = constp.tile([P, KC, P], bf16, name="sa_head")
            wq_r = wq_d.ap().rearrange("(ko p) a m -> p ko a m", p=P)
            wk_r = wk_d.ap().rearrange("(ko p) a m -> p ko a m", p=P)

            # persistent activations
            # qT/kT: [4 heads x 32 depth on partitions, A/B k-tile, cols]
            qT_sb = persist.tile([P, 2, F], f8e4, name="qT_sb")
            kT_sb = persist.tile([P, 2, T], f8e4, name="kT_sb")
            v_sb = persist.tile([P, NTT, NHEADS, H], bf16, name="v_sb")
            attnT_sb = persist.tile([P, 2, F], bf16, name="attnT_sb")
            nc.vector.memset(ones1[:], 1.0)

            qT_r = qT_d.ap().rearrange("(ko p) f -> p ko f", p=P)
            sT_r = sT_d.ap().rearrange("(ko p) t -> p ko t", p=P)
            sa_tiles = {}
            qa_tiles = {}
            qps = {}

            # ---- projections (piecewise emission) ----
            def q_dma(fc, ks=None):
                fsl = slice(fc * fc_w, (fc + 1) * fc_w)
                qa = qap.tile([P, KC, fc_w], bf16, tag="qa", name="qa")
                qa_tiles[fc] = qa
                if ks is None:
                    nc.sync.dma_start(qa[:], qT_r[:, :, fsl])
                else:
                    for k in ks:
                        nc.sync.dma_start(qa[:, k, :], qT_r[:, k, fsl])

            def q_piece(fc, a, half, pool):
                # half 0: k=0..3 (allocates psq), half 1: k=4..7 + copy
                fsl = slice(fc * fc_w, (fc + 1) * fc_w)
                qa = qa_tiles[fc]
                if half == 0:
                    qps[(fc, a)] = pool.tile([P, 512], f32, tag="bank", name="psq")
                psq = qps[(fc, a)]
                for k in range(4 * half, 4 * half + 4):
                    nc.tensor.matmul(
                        psq[:, :fc_w], lhsT=wq_sb[:, k, a, :], rhs=qa[:, k, :],
                        start=(k == 0), stop=(k == KC - 1))
                if half == 1:
                    nc.vector.tensor_copy(qT_sb[:, a, fsl], psq[:, :fc_w])
                    del qps[(fc, a)]

            def k_proj(sc, pool, csl=slice(0, fc_w), src_head=False,
                       a1_act=False):
                ssl = slice(sc * fc_w + csl.start, sc * fc_w + csl.stop)
                sa = sa_tiles[sc]
                n = csl.stop - csl.start
                for a in range(2):
                    psk = pool.tile([P, 512], f32, tag="bank", name="psk")
                    for k in range(KC):
                        rhs = sa_head[:, k, csl] if src_head else sa[:, k, csl]
                        nc.tensor.matmul(
                            psk[:, :n], lhsT=wk_sb[:, k, a, :], rhs=rhs,
                            start=(k == 0), stop=(k == KC - 1))
                    if a == 1 and a1_act:
                        nc.scalar.copy(kT_sb[:, a, ssl], psk[:, :n])
                    else:
                        nc.vector.tensor_copy(kT_sb[:, a, ssl], psk[:, :n])

            def v_piece(tt, pool):
                sc, tl = tt // FPC, tt % FPC
                sa = sa_tiles[sc]
                psv = pool.tile([P, 512], f32, tag="bank", name="psv")
                for k in range(KC):
                    lhsT = (sa_head[:, k, :] if tt == 0
                            else sa[:, k, tl * P:(tl + 1) * P])
                    nc.tensor.matmul(
                        psv[:, :NH], lhsT=lhsT,
                        rhs=wv_sb[:, k, :],
                        start=(k == 0), stop=(k == KC - 1))
                nc.vector.tensor_copy(
                    v_sb[:, tt, :, :],
                    psv[:, :NH].rearrange("p (h x) -> p h x", h=NHEADS))

            # ---- softmax stream ----
            pt_store = {}

            mul_args = {}

            def produce(fc, tt, mul=True):
                fsl = slice(fc * fc_w, (fc + 1) * fc_w)
                tsl = slice(tt * P, (tt + 1) * P)
                bias_t = biasp.tile([P, fc_w], bf16, tag="bias", name="bias_t")
                nc.sync.dma_start(bias_t[:], eb_d.ap()[tsl, fsl])
                pt4 = ptp.tile([P, NHEADS, fc_w], bf16, tag="pt", name="pt4")
                for pair in range(2):
                    st2 = psS.tile([P, 2, 512], f32, tag="st", name="st2")
                    for j in range(2):
                        h = 2 * pair + j
                        nc.tensor.matmul(
                            st2[:, j, :fc_w],
                            lhsT=kT_sb[32 * h:32 * h + 32, :, tsl],
                            rhs=qT_sb[32 * h:32 * h + 32, :, fsl],
                            start=True, stop=True,
                            perf_mode=DR, tile_position=(32 * h, 0))
                    nc.scalar.activation(
                        pt4[:, 2 * pair:2 * pair + 2, :], st2[:, :, :fc_w],
                        Exp, scale=ESCALE)
                pt_store[(fc, tt)] = pt4
                mul_args[(fc, tt)] = (pt4, bias_t)
                if mul:
                    produce_mul(fc, tt)

            def produce_mul(fc, tt, dve_only=False):
                # *exp(bias). Normally heads 0-2 on DVE (2x mode) + head 3 on
                # GPSIMD; dve_only puts all 4 heads in one DVE instruction
                # (used for chunk 0 so its deferred muls don't flood Pool).
                pt4, bias_t = mul_args.pop((fc, tt))
                if dve_only == "split":
                    for hp in range(2):
                        nc.vector.tensor_mul(
                            pt4[:, 2 * hp:2 * hp + 2, :],
                            pt4[:, 2 * hp:2 * hp + 2, :],
                            bias_t[:, None, :].to_broadcast((P, 2, fc_w)))
                elif dve_only:
                    nc.vector.tensor_mul(
                        pt4[:], pt4[:],
                        bias_t[:, None, :].to_broadcast((P, NHEADS, fc_w)))
                else:
                    nc.vector.tensor_mul(
                        pt4[:, 0:3, :], pt4[:, 0:3, :],
                        bias_t[:, None, :].to_broadcast((P, 3, fc_w)))
                    nc.gpsimd.tensor_tensor(
                        pt4[:, 3, :], pt4[:, 3, :], bias_t[:], Mult)

            # ---- attention accumulate ----
            acc = {}

            def alloc_acc(fc):
                vb = []
                for b in range(2):
                    raw = vaccp.tile([P, 512], f32, tag="bank", name=f"vacc{b}")
                    vb.append(raw.rearrange("p (r h x) -> p r h x", r=2, h=NHEADS))
                dn_raw = denp.tile([P, 512], f32, tag="den", name="dn")
                dn = dn_raw[:, :FPC * NHEADS].rearrange(
                    "p (fl h) -> p fl h", fl=FPC)
                acc[fc] = (vb, dn)

            def consume(fc, tt):
                # One PSUM epoch per bank and chunk: only the first write
                # carries start (bumps the bank epoch; stale-tag writes then
                # overwrite), only the last carries stop.
                pt4 = pt_store.pop((fc, tt))
                vb, dn = acc[fc]
                if fc == NFC - 1 and tt == NTT - 1:
                    # end of the very last accumulation: denominators first
                    # (h-major, tracking the split bias-mul halves) so the
                    # reciprocal starts ~0.4us earlier; value matmuls after.
                    for h in range(NHEADS):
                        for fl in range(FPC):
                            nc.tensor.matmul(
                                dn[:, fl, h:h + 1],
                                lhsT=pt4[:, h, fl * P:(fl + 1) * P],
                                rhs=ones1[:], start=False,
                                stop=(fl == FPC - 1 and h == NHEADS - 1))
                    for h in range(NHEADS):
                        for fl in range(FPC):
                            b, r = fl // 2, fl % 2
                            nc.tensor.matmul(
                                vb[b][:, r, h, :],
                                lhsT=pt4[:, h, fl * P:(fl + 1) * P],
                                rhs=v_sb[:, tt, h, :], start=False,
                                stop=(fl == FPC - 1 and h == NHEADS - 1))
                    return
                for fl in range(FPC):
                    b, r = fl // 2, fl % 2
                    lhs = pt4[:, :, fl * P:(fl + 1) * P]
                    for h in range(NHEADS):
                        nc.tensor.matmul(
                            vb[b][:, r, h, :], lhsT=lhs[:, h, :],
                            rhs=v_sb[:, tt, h, :],
                            start=(tt == 0 and r == 0 and h == 0),
                            stop=(tt == NTT - 1 and r == 1 and h == NHEADS - 1))
                        nc.tensor.matmul(
                            dn[:, fl, h:h + 1], lhsT=lhs[:, h, :],
                            rhs=ones1[:],
                            start=(tt == 0 and fl == 0 and h == 0),
                            stop=(tt == NTT - 1 and fl == FPC - 1
                                  and h == NHEADS - 1))

            # ---- finish (normalize / transpose / output projection) ----
            flats = {}

            def fin_normalize(fc):
                vb, dn = acc.pop(fc)
                flat = flatp.tile([P, FPC, NHEADS, H], f32, tag="flat", name="flat")
                flats[fc] = flat
                rec = smallp.tile([P, FPC, NHEADS, 1], f32, tag="rec", name="rec")
                nc.vector.reciprocal(rec[:], dn[:, :, :, None])
                for b in range(2):
                    nc.vector.tensor_tensor(
                        flat[:, 2 * b:2 * b + 2, :, :], vb[b][:],
                        rec[:, 2 * b:2 * b + 2, :, :].to_broadcast(
                            (P, 2, NHEADS, H)), Mult)

            def fin_tr(fc, m, pool):
                fsl = slice(fc * fc_w, (fc + 1) * fc_w)
                flat = flats[fc]
                tr = pool.tile([P, 512], f32, tag="bank", name="tr")
                for fl in range(FPC):
                    nc.tensor.transpose(
                        tr[:, fl * P:(fl + 1) * P],
                        flat[:, fl, 2 * m:2 * m + 2, :], ident[:])
                nc.vector.tensor_copy(attnT_sb[:, m, fsl], tr[:])
                if m == 1:
                    del flats[fc]

            def fin_pso(fc, piece, pool, stage="dve"):
                # piece = (fl, cc): one 128F x 512C output block
                fl, cc = piece // 2, piece % 2
                ft = fc * FPC + fl
                pso = pool.tile([P, 512], f32, tag="bank", name="pso")
                for m in range(2):
                    nc.tensor.matmul(
                        pso[:], lhsT=attnT_sb[:, m, ft * P:(ft + 1) * P],
                        rhs=wo_sb[:, m, cc * 512:(cc + 1) * 512],
                        start=(m == 0), stop=(m == 1))
                ot = outp.tile([P, 512], f32, tag="o", name="ot")
                if stage == "act":
                    nc.scalar.copy(ot[:], pso[:])
                else:
                    nc.vector.tensor_copy(ot[:], pso[:])
                nc.sync.dma_start(
                    out_d.ap()[ft * P:(ft + 1) * P, cc * 512:(cc + 1) * 512],
                    ot[:])

            # ================= schedule =================
            # Prefix: split first-chunk DMAs per k-tile so PE starts ~2us in;
            # first exp as soon as q-chunk 0 + first k T-tile are projected.
            qa0 = qap.tile([P, KC, fc_w], bf16, tag="qa", name="qa")
            qa_tiles[0] = qa0
            sa0 = sap.tile([P, KC, fc_w], bf16, tag="sa", name="sa")
            sa_tiles[0] = sa0
            # p-state warmup: keep PE continuously busy on junk matmuls from
            # ~1us so the ramp (full speed after 3us busy) completes before
            # the real q-projection matmuls arrive.
            junk = smallp.tile([P, 512], bf16, tag="junk", bufs=1, name="junk")
            nc.vector.memset(junk[:], 1.0)
            for w in range(8):
                psw = vaccp.tile([P, 512], f32, tag="bank", name="psw")
                nc.tensor.matmul(psw[0:1, 0:512], lhsT=ones1[:], rhs=junk[:],
                                 start=True, stop=True)
            # halved transfers ordered so the q matmuls start ~3us in; the
            # first k projection needs only wk + the first 128 T columns.
            # DMA issue order = transfer order on the serial DMA resource:
            # wk + the prepacked first T-tile first (k projection has the
            # longer post-DMA chain, PE processes it behind the junk warmup),
            # then qa/wq per k-pair so the q matmuls track arrival; sa0's
            # remainder, wv, sa1 follow so exp(0,1..4)'s k chains beat the
            # ACT pace; wo and ident (iteration 2) are deferred into the sc
            # loop so they never sit in front of sa2.
            nc.sync.dma_start(wk_sb[:], wk_r)
            # first 128 T-cols arrive via the host-prepacked contiguous sth
            # tensor: one 2KB/partition run (the strided sT slice would pay
            # the cost model's 2x small-run DMA penalty and take 2x longer).
            nc.sync.dma_start(
                sa_head[:].rearrange("p ko t -> p (ko t)"), sth_d.ap())
            for kq in range(4):
                ks = slice(2 * kq, 2 * kq + 2)
                nc.sync.dma_start(qa0[:, ks, :], qT_r[:, ks, 0:fc_w])
                nc.sync.dma_start(wq_sb[:, ks], wq_r[:, ks])
            nc.sync.dma_start(sa0[:, :, P:3 * P], sT_r[:, :, P:3 * P])
            nc.sync.dma_start(sa0[:, :, 3 * P:fc_w], sT_r[:, :, 3 * P:fc_w])
            nc.sync.dma_start(wv_sb[:], wv_d.ap().rearrange("(ko p) m -> p ko m", p=P))
            sa1 = sap.tile([P, KC, fc_w], bf16, tag="sa", name="sa")
            nc.sync.dma_start(sa1[:], sT_r[:, :, fc_w:2 * fc_w])
            sa_tiles[1] = sa1
            k_proj(0, vaccp, slice(0, P), src_head=True, a1_act=True)
            # chunk-0 q projection: both a-halves' PSUM banks accumulate in
            # k-pair order (tracking the split DMAs), copies on DVE.
            psq0 = vaccp.tile([P, 512], f32, tag="bank", name="psq")
            psq1 = vaccp.tile([P, 512], f32, tag="bank", name="psq")
            for k in range(KC):
                for a, psq in ((0, psq0), (1, psq1)):
                    nc.tensor.matmul(
                        psq[:], lhsT=wq_sb[:, k, a, :], rhs=qa0[:, k, :],
                        start=(k == 0), stop=(k == KC - 1))
            nc.vector.tensor_copy(qT_sb[:, 0, 0:fc_w], psq0[:])
            nc.scalar.copy(qT_sb[:, 1, 0:fc_w], psq1[:])
            # k piece 2 goes before the first produce: PE is otherwise idle
            # between the q matmuls (whose copies run on DVE) and the first
            # scores, and exp(0,1)'s chain needs the piece done early.
            k_proj(0, vaccp, slice(P, 2 * P))
            produce(0, 0, mul=False)
            k_proj(0, vaccp, slice(2 * P, 3 * P))
            next_v = 0
            for tt in range(1, FPC):
                produce(0, tt, mul=False)
                if tt == 1:
                    k_proj(0, vaccp, slice(3 * P, fc_w))
                else:
                    v_piece(next_v, vaccp); next_v += 1
            # chunk-0 bias muls (DVE-only) are deferred TWO source chunks so
            # they never sit in front of the k/q copies the score stream
            # needs next; q1's projection rides the sc2/sc3 slack slots so
            # its copies are long done before iteration 1 starts.
            q_dma(1)
            for sc in range(1, T // fc_w):
                # issue the NEXT source chunk's DMA one iteration early so
                # its hoisted first k-piece (emitted at this chunk's last
                # slot, ahead of the sc-boundary PE backlog) has data.
                if sc <= 2:
                    sa = sap.tile([P, KC, fc_w], bf16, tag="sa", name="sa")
                    nc.sync.dma_start(
                        sa[:], sT_r[:, :, (sc + 1) * fc_w:(sc + 2) * fc_w])
                    sa_tiles[sc + 1] = sa
                    if sc == 1:
                        nc.sync.dma_start(
                            wo_sb[:], wo_d.ap().rearrange("(ko p) m -> p ko m", p=P))
                    else:
                        nc.sync.dma_start(ident[:], id_d.ap())
                for tl in range(FPC):
                    tt = sc * FPC + tl
                    if tl == 0:
                        if sc == 1:
                            k_proj(sc, vaccp, slice(0, P))
                        produce(0, tt, mul=False)
                        k_proj(sc, vaccp, slice(P, 2 * P))
                    else:
                        produce(0, tt, mul=False)
                    if tl == 1:
                        k_proj(sc, vaccp, slice(2 * P, fc_w))
                        if sc >= 2:
                            for mt in range(4 * (sc - 2), 4 * sc - 4):
                                produce_mul(0, mt, dve_only=True)
                    elif tl >= 2:
                        if sc == 1:
                            v_piece(next_v, vaccp); next_v += 1
                        else:
                            q_piece(1, sc - 2, tl - 2, vaccp)
                        if tl == 3 and sc <= 2:
                            k_proj(sc + 1, vaccp, slice(0, P))
            v_piece(next_v, vaccp); next_v += 1
            v_piece(next_v, vaccp); next_v += 1
            v_piece(next_v, vaccp); next_v += 1
            for mt in range(8, NTT):
                produce_mul(0, mt, dve_only=True)

            # Iteration 1: produce chunk 1, consume chunk 0 (1/slot);
            # late v pieces and q2 quarters in the slack.
            alloc_acc(0)
            for tt in range(NTT):
                produce(1, tt)
                if tt <= 9:
                    v_piece(tt + 6, finp)
                if tt == 10:
                    q_dma(2)
                if 10 <= tt <= 13:
                    q_piece(2, (tt - 10) // 2, (tt - 10) % 2, finp)
                if tt == 14:
                    q_dma(3)
                consume(0, tt)
            fin_normalize(0)

            # Iteration 2: produce 2, consume 1 (1/slot); finish(0) pieces
            # + q3 quarters in the slack.
            alloc_acc(1)
            c1 = 0
            for tt in range(NTT):
                produce(2, tt)
                if tt <= 1:
                    q_piece(3, tt, 0, finp)
                    q_piece(3, tt, 1, finp)
                elif tt in (2, 3):
                    fin_tr(0, tt - 2, finp)
                elif 4 <= tt <= 11:
                    fin_pso(0, tt - 4, finp)
                elif tt in (12, 13):
                    fin_tr(1, tt - 12, finp)
                else:
                    fin_pso(1, tt - 14, finp)
                if tt <= 7:
                    consume(1, c1); consume(1, c1 + 1); c1 += 2
                elif tt == 8:
                    fin_normalize(1)

            # Iteration 3: produce 3; consume 2 compressed 2/slot so chunk 3
            # can start consuming mid-iteration; finish(1) then finish(2)
            # pieces in the slack.
            alloc_acc(2)
            c2 = 0
            c3 = 0
            for tt in range(NTT):
                # last two tiles: DVE-only mul keeps Pool's ~1.2us latency
                # out of the tail's critical chain into consume(3,15)
                produce(3, tt, mul=(tt < 14))
                if tt >= 14:
                    produce_mul(3, tt, dve_only="split" if tt == 15 else True)
                if tt <= 5:
                    fin_pso(1, 2 + tt, finp)
                if tt <= 7:
                    consume(2, c2); consume(2, c2 + 1); c2 += 2
                elif tt == 8:
                    fin_normalize(2)
                    alloc_acc(3)
                    consume(3, c3); c3 += 1
                elif tt in (9, 10):
                    fin_tr(2, tt - 9, finp)
                    consume(3, c3); c3 += 1
                else:
                    fin_pso(2, tt - 11, finp)
                    consume(3, c3); consume(3, c3 + 1); c3 += 2
            # Tail: remaining chunk-3 consumes, last finishes. finish(3)
            # runs through the (now free) score banks, finish(2) leftovers
            # through the finish bank concurrently.
            while c3 < NTT:
                consume(3, c3); c3 += 1
            # ACT staging: these three run inside the tail's DVE-critical
            # window (mul/recip/flat); ACT is idle until the attnT copies
            fin_pso(2, 5, finp, stage="act")
            fin_pso(2, 6, finp, stage="act")
            fin_pso(2, 7, finp, stage="act")
            fin_normalize(3)
            # last chunk: all four F-tiles in flight at once across every
            # freed PSUM bank. Each (fl, m) transpose gets its own slot -
            # sharing a slot would serialize the second transpose behind the
            # first's attnT copy (per-tile epoch tracking); pso(fl, cc)
            # reuses slot 2*fl+cc whose transpose it already depends on
            # through attnT. Staging copies alternate ACT/DVE and each
            # 512-col half DMAs as soon as it lands.
            flat3 = flats.pop(3)
            slots = []   # eight 1-bank [P, 512] psum slots
            for b in range(2):
                t = psS.tile([P, 2, 512], f32, tag="st", name="trps")
                slots.extend([t[:, 0, :], t[:, 1, :]])
            slots.append(vaccp.tile([P, 512], f32, tag="bank", name="pv")[:])
            slots.append(vaccp.tile([P, 512], f32, tag="bank", name="pv")[:])
            slots.append(denp.tile([P, 512], f32, tag="den", name="pd")[:])
            slots.append(finp.tile([P, 512], f32, tag="bank", name="pf")[:])
            for fl in range(FPC):
                ft = (NFC - 1) * FPC + fl
                fcols = slice((NFC - 1) * fc_w + fl * P,
                              (NFC - 1) * fc_w + (fl + 1) * P)
                for m in range(2):
                    tr = slots[(2 * fl + m) % 8]
                    nc.tensor.transpose(
                        tr[:, 0:P], flat3[:, fl, 2 * m:2 * m + 2, :], ident[:])
                    if m == 0:
                        nc.scalar.copy(attnT_sb[:, m, fcols], tr[:, 0:P])
                    else:
                        nc.vector.tensor_copy(attnT_sb[:, m, fcols],
                                              tr[:, 0:P])
                ot2 = outp.tile([P, 2, 512], f32, tag="o2", bufs=4, name="ot2")
                for cc in range(2):
                    po = slots[(2 * fl + cc) % 8]
                    for m in range(2):
                        nc.tensor.matmul(
                            po[:],
                            lhsT=attnT_sb[:, m, ft * P:(ft + 1) * P],
                            rhs=wo_sb[:, m, cc * 512:(cc + 1) * 512],
                            start=(m == 0), stop=(m == 1))
                    if (fl + cc) % 2 == 1:
                        nc.scalar.copy(ot2[:, cc, :], po[:])
                    else:
                        nc.vector.tensor_copy(ot2[:, cc, :], po[:])
                    nc.sync.dma_start(
                        out_d.ap()[ft * P:(ft + 1) * P,
                                   cc * 512:(cc + 1) * 512],
                        ot2[:, cc, :])


    nc.compile()
    return nc


_CACHE = {}


def _get_nc():
    if "nc" not in _CACHE:
        _CACHE["nc"] = build_attention_nc(C=C, F=F, T=T, NHEADS=HEADS // HG, H=DEPTH)
    return _CACHE["nc"]


def kernel(query_input, source_input, bias, wq, wk, wv, wo, **run_kwargs):
    import ml_dtypes
    from concourse.bass_utils import run_bass_kernel_spmd

    bf = ml_dtypes.bfloat16
    q = np.asarray(query_input, dtype=np.float32)
    s = np.asarray(source_input, dtype=np.float32)
    b = np.asarray(bias, dtype=np.float32)
    wq4 = np.asarray(wq, dtype=np.float32) * WSC   # [C, 16, 64]
    wk4 = np.asarray(wk, dtype=np.float32) * WSC
    wv4 = np.asarray(wv, dtype=np.float32) * WSC
    wo4 = np.asarray(wo, dtype=np.float32) / WSC   # [16, 64, C]

    qT = [np.ascontiguousarray(q[i].T).astype(bf) for i in range(B)]
    sT = [np.ascontiguousarray(s[i].T).astype(bf) for i in range(B)]
    # first 128 T-cols prepacked contiguous per partition: [p, ko, t] with
    # element (p, ko, t) = sT[ko*128 + p, t]
    sth = [np.ascontiguousarray(
        s[i].T[:, :128].reshape(8, 128, 128).transpose(1, 0, 2)
        .reshape(128, 1024)).astype(bf) for i in range(B)]
    ebT = np.exp(np.ascontiguousarray(b[0, 0].T)).astype(bf)
    ident = np.eye(128, dtype=np.float32)

    in_maps = []
    for c in range(N_CORES):
        bi, hg = c // HG, c % HG
        hsl = slice(hg * HG, (hg + 1) * HG)
        # wq/wk: [C, 2(A|B), 4 heads, 32 depth] -> [C, 2, 128]
        wqg = wq4[:, hsl, :].reshape(C, HG, 2, 32).transpose(0, 2, 1, 3)
        wkg = wk4[:, hsl, :].reshape(C, HG, 2, 32).transpose(0, 2, 1, 3)
        in_maps.append({
            "qT": qT[bi],
            "sT": sT[bi],
            "sth": sth[bi],
            "ebT": ebT,
            "wq": np.ascontiguousarray(wqg.reshape(C, 2, 128)).astype(bf),
            "wk": np.ascontiguousarray(wkg.reshape(C, 2, 128)).astype(bf),
            "wv": np.ascontiguousarray(
                wv4[:, hsl, :].reshape(C, HG * DEPTH)).astype(bf),
            "wo": np.ascontiguousarray(
                wo4[hsl, :, :].reshape(HG * DEPTH, C)).astype(bf),
            "ident": ident,
        })

    nc = _get_nc()
    res = run_bass_kernel_spmd(nc, in_maps, core_ids=list(range(N_CORES)), **run_kwargs)
    _CACHE["last_results"] = res

    out = np.empty((B, F, C), np.float32)
    for bi in range(B):
        acc = res.results[bi * HG]["out_p"].astype(np.float32)
        for hg in range(1, HG):
            acc = acc + res.results[bi * HG + hg]["out_p"]
        out[bi] = acc
    return out


# ---------------------------------------------------------------------------
# Performance state (cost-model timeline, per core): 159201 ns.
# Progression: 167610 (v3) -> 164032 (DMA issue-order fixes: wv+sa1 early,
# wo/ident deferred, sa0 as >=512B-run pieces) -> 163101 (head package:
# host-prepacked contiguous sth tensor for the first T-tile at 728ns instead
# of the 2x small-run penalty; k-first emission; chunk-0 q matmuls
# interleaved per k-pair DMA arrival; ACT carries the a1 qT/kT copies while
# idle pre-first-exp) -> 161270 (prefix k-piece interleave, split dve-only
# mul for the last tile, 8-slot tail with per-(fl,m) transpose tiles) ->
# 159201 (sa(sc+1) DMA + first k-piece hoisted into chunk sc's last slot:
# the mid-kernel exp stream is now gap-free from exp(0,2) to exp(3,15)).
# Validated on 8 NeuronCores at relative error 1.7033e-02 (gate 2e-2).
#
# HARD-WON HARDWARE FACTS (cost model/CoreSim/birsim do NOT catch these;
# only a device run fails, with an opaque INTERNAL error at readback):
# - fp8 (e4m3) matmuls, DR or plain, MUST write PSUM at bank offset 0.
#   A 256-f32 output region at byte offset 1024 inside a bank kills the run.
#   bf16 matmuls (value-accumulate, denominators) and f32 transposes may
#   write offset regions freely. This is why 256-wide F-chunks are
#   impractical: 4 heads x 256 scores cannot be packed bank-aligned into
#   2 banks, and bank-aligned layouts force N=512 exps (+5.9us ACT).
# - GPSIMD (Pool) cannot access PSUM at all (BIR verifier rejects).
# - ACT (scalar.copy) writing f8e4 output is fine on HW.
#
# Where the remaining ~22us above the 137us ACT floor lives:
# - Head ~13us: serial DMA (wk+sth+qa+wq = 8.5us incl 2us start latency)
#   then the PE-dispatch-bound chain q-matmuls -> copies -> scores; exp(0,1)
#   slips ~1.2us (PE in-order, 4 k-pieces + q + scores don't quite fit).
# - Tail ~14us after the last exp: split mul -> consume(3,15) -> normalize
#   -> transposes/attnT copies (parallel ACT/DVE) -> 16 out-proj matmuls
#   (PE-serial 3.4us) -> staging copies -> 8x728ns serial out-DMAs + 1.6us
#   drain. Narrowing the last chunk would halve this but is blocked by the
#   fp8 PSUM alignment erratum (see above).
# - Iteration-2 compression (consume(1) 2/slot, fin(1) pulled forward) and
#   the sc-boundary k-piece hoist bought ~5us; the exp stream has zero gaps
#   >200ns between 12us and 145us.
# ---------------------------------------------------------------------------
# - sth2 prepack (source-chunk-1 first T-tile contiguous) + the 11-DMA head:
#   first exp 12131 -> 11455 and the exp(0,1) slip 1181 -> 559, but new
#   447/522ns stalls appear at exp(0,4)/(0,6) (prefix v-pieces waiting wv
#   block the sc1 k-pieces on the in-order PE; the earlier exp stream
#   tightens every downstream chain). Net +415ns. The sth2 host plumbing
#   and k_proj(src_head=2)/v_piece(tt==4) wiring are straightforward if a
#   future session wants to re-balance the sc1-era PE queue around it.
# ---------------------------------------------------------------------------
# Session-5 probes (all regressed; 158984 is a verified local optimum):
# - Split late-block stagings into ACT||DVE halves: +178 (doubled queue ops).
# - All-8-transposes-first fan-out: exactly neutral (scheduler already
#   achieves the interleave).
# - fin(2) leftovers hoisted before the last consumes: +651 (their pso
#   matmuls push consume(3,14/15) back on the in-order PE).
# ---------------------------------------------------------------------------
# - k_proj(0,P:2P) after produce(0,0): first exp 12131 -> 11509 but a new
#   598ns stall at exp(0,6) (sc1 k-pieces pushed back); net +281.
# ---------------------------------------------------------------------------
